# revision 33
# baseline (speedup 1.0000x reference)
"""Trainium2 Bass kernel for nn_AttentiveSSM (sparse chunked attention + SSM).

Sharding (8 cores, tensor-parallel over heads):
  core c owns q-heads {2c, 2c+1} and kv-head c//2. Each core computes its
  Q/K/V projections from the full (transposed) x in bf16, runs the chunked
  SSM + RoPE, sparse attention against the compressed key set (chunk
  boundaries + first-4 + t-1 diagonal), and a partial output projection
  through its wo column slice. Host sums the 8 partial yT outputs.

v2: bf16 matmul pipeline, stacked score/exp tiles, paired-head diag chain,
PE-dense issue order (proj/scores/tails/wo interleave), PSUM bank plan:
  A: q0/q1 + wo ring   B: k/v + wo ring   C: vc transposes + wo ring
  sc ring(2): score stacks, st4, psb, rb   sm(1): sd/den pairs   oun(2)

Self-contained: hardcodes all shapes; no sibling imports.
"""
import sys
import numpy as np

sys.path.insert(0, '/opt/trn_rl_repo')

import concourse.bacc as bacc               # noqa: E402
import concourse.mybir as mybir             # noqa: E402
from concourse.tile import TileContext      # noqa: E402
from concourse import bass_utils            # noqa: E402
from concourse.alu_op_type import AluOpType # noqa: E402

# silence cloud artifact upload in traced runs
bass_utils.upload_artifacts = lambda tmpdir: tmpdir

S = 2048          # sequence
D = 2048          # model dim
HD = 128          # head dim
QB = 512          # query block
NSB = S // QB     # 4 s-blocks
NKT = D // 128    # 16 contraction tiles
KC = 8            # token chunk
SCALE = float(1.0 / np.sqrt(HD))

F32 = mybir.dt.float32
F32R = mybir.dt.float32r
BF = mybir.dt.bfloat16
MUL = AluOpType.mult
ADD = AluOpType.add
EXP = mybir.ActivationFunctionType.Exp

_CACHE = {}


def _build_module():
    nc = bacc.Bacc("TRN2", num_devices=8)

    def din(name, shape, dt):
        return nc.dram_tensor(name, list(shape), dt, kind="ExternalInput")

    xT = din("xT", (D, S), BF)
    wqT = din("wqT", (D, 256), BF)
    wkT = din("wkT", (D, 128), BF)
    wvT = din("wvT", (D, 128), BF)
    woT0 = din("woT0", (128, D), BF)
    woT1 = din("woT1", (128, D), BF)
    cosk = din("cosk", (128, S), BF)    # halves duplicated
    sink = din("sink", (128, S), BF)    # [sin; -sin]
    akp = din("akp", (128, QB), F32)    # scan decay pattern (0 at i%8==0)
    avp = din("avp", (128, QB), F32)
    cbk = din("cbk", (128, 1), F32)     # c*b fused SSM output scale
    cbv = din("cbv", (128, 1), F32)
    bandm01 = din("bandm01", (128, QB), BF)  # post-exp staircase mask, rows
                                             # duplicated so any 64-partition
                                             # slice is partition-aligned
    f4m01 = din("f4m01", (4, QB), BF)
    mdiag01 = din("mdiag01", (4, QB), BF)    # diag valid mask per block
    identb = din("identb", (128, 128), BF)
    onesb = din("onesb", (128, 1), BF)
    onesrowb = din("onesrowb", (1, 128), BF)
    onesrowf = din("onesrowf", (1, 128), F32)
    yT = nc.dram_tensor("yT", [D, S], BF, kind="ExternalOutput")

    with TileContext(nc) as tc:
        with (
            tc.tile_pool(name="const", bufs=1) as cp,
            tc.tile_pool(name="big", bufs=1) as bp,
            tc.tile_pool(name="xs", bufs=20) as xs,
            tc.tile_pool(name="tmp", bufs=2) as tp,
            tc.tile_pool(name="psA", bufs=1, space="PSUM") as psA,
            tc.tile_pool(name="psB", bufs=1, space="PSUM") as psB,
            tc.tile_pool(name="psC", bufs=1, space="PSUM") as psC,
            tc.tile_pool(name="scp", bufs=3, space="PSUM") as scp,
            tc.tile_pool(name="ounp", bufs=2, space="PSUM") as ounp,
        ):
            # ---- constant tiles ----
            def cload(name, shape, src, dt, eng):
                t = cp.tile(list(shape), dt, tag=name, name=name)
                eng.dma_start(t[:], src[:])
                return t

            # Act queue: wq then wk (wv issued later, after cast-q0(0))
            wq_sb = cp.tile([128, NKT * 256], BF, tag="wq")
            wk_sb = cp.tile([128, NKT * 128], BF, tag="wk")
            wv_sb = cp.tile([128, NKT * 128], BF, tag="wv")
            for k in range(NKT):
                nc.scalar.dma_start(wq_sb[:, k * 256:(k + 1) * 256],
                                    wqT[128 * k:128 * (k + 1), :])
            for k in range(NKT):
                nc.scalar.dma_start(wk_sb[:, k * 128:(k + 1) * 128],
                                    wkT[128 * k:128 * (k + 1), :])
            # SP queue: SSM consts (x tiles issued below, first)
            # gpsimd queue: rope tables, masks, ident, wo weights (late use)
            cosk_s = cload("cosk", (128, S), cosk, BF, nc.gpsimd)
            sink_s = cload("sink", (128, S), sink, BF, nc.gpsimd)
            identb_s = cload("identb", (128, 128), identb, BF, nc.gpsimd)
            onesb_s = cload("onesb", (128, 1), onesb, BF, nc.gpsimd)
            onesrowb_s = cload("onesrowb", (1, 128), onesrowb, BF, nc.gpsimd)
            onesrowf_s = cload("onesrowf", (1, 128), onesrowf, F32, nc.gpsimd)
            bandm01_s = cload("bandm01", (128, QB), bandm01, BF, nc.gpsimd)
            f4m01_s = cload("f4m01", (4, QB), f4m01, BF, nc.gpsimd)
            mdiag01_s = [cload(f"mdiag01_{b}", (1, QB), mdiag01[b:b + 1, :],
                               BF, nc.gpsimd) for b in range(4)]
            wo_sb0 = cload("wo0", (128, D), woT0, BF, nc.gpsimd)
            wo_sb1 = cload("wo1", (128, D), woT1, BF, nc.gpsimd)

            # ---- big persistent state ----
            QT0 = bp.tile([128, S], BF, tag="QT0")
            QT1 = bp.tile([128, S], BF, tag="QT1")
            KTp = bp.tile([128, S], BF, tag="KTp")
            VTp = bp.tile([128, S], BF, tag="VTp")
            OT0 = bp.tile([128, S], BF, tag="OT0")
            OT1 = bp.tile([128, S], BF, tag="OT1")
            KCt = bp.tile([128, 260], BF, tag="KCt")   # [b0|b1|b2|b3|first4]
            VG = bp.tile([128, 260], BF, tag="VG")
            vstk0 = bp.tile([128, 128], BF, tag="vstk0")  # keys 0:128 (hd-major)
            vstk1 = bp.tile([128, 128], BF, tag="vstk1")  # keys 128:256
            vc4 = bp.tile([4, 128], BF, tag="vc4")

            # x tile handles per block: (tile, col offset)
            xtiles = [[None] * NKT for _ in range(NSB)]

            def load_x(b):
                s0 = QB * b
                for k in range(NKT):
                    xt = xs.tile([128, QB], BF, tag="x", name=f"x{b}_{k}")
                    nc.sync.dma_start(xt[:], xT[128 * k:128 * (k + 1),
                                                s0:s0 + QB])
                    xtiles[b][k] = (xt, 0)

            def load_x2(b):
                # batched pair: one [128, 2*QB] DMA covers s-blocks b, b+1
                s0 = QB * b
                for k in range(NKT):
                    xt = xs.tile([128, 2 * QB], BF, tag="x2", bufs=16,
                                 name=f"x2{b}_{k}")
                    nc.sync.dma_start(xt[:], xT[128 * k:128 * (k + 1),
                                                s0:s0 + 2 * QB])
                    xtiles[b][k] = (xt, 0)
                    xtiles[b + 1][k] = (xt, QB)

            # SP queue: x(0) head start, SSM consts, rest of x(0), x(1)
            for k in range(4):
                xt = xs.tile([128, QB], BF, tag="x", name=f"x0_{k}")
                nc.sync.dma_start(xt[:], xT[128 * k:128 * (k + 1), 0:QB])
                xtiles[0][k] = (xt, 0)
            akp_s = cload("akp", (128, QB), akp, F32, nc.sync)
            avp_s = cload("avp", (128, QB), avp, F32, nc.sync)
            cbk_s = cload("cbk", (128, 1), cbk, F32, nc.sync)
            cbv_s = cload("cbv", (128, 1), cbv, F32, nc.sync)
            for k in range(4, NKT):
                xt = xs.tile([128, QB], BF, tag="x", name=f"x0_{k}")
                nc.sync.dma_start(xt[:], xT[128 * k:128 * (k + 1), 0:QB])
                xtiles[0][k] = (xt, 0)
            load_x(1)

            # per-block psum/sbuf handles threaded between phases
            ps = [dict() for _ in range(NSB)]

            def proj(b):
                """QKV projection matmuls + q casts. PE order: q0, k, q1, v.
                Bank A: q0 then q1 (q1 waits cast-q0). Bank B: k then v
                (v waits scan/stt-k reading psum k)."""
                def xap(k):
                    xt, off = xtiles[b][k]
                    return xt[:, off:off + QB]
                psq0 = psA.tile([128, QB], F32, tag="A", name=f"q0_{b}")
                for k in range(NKT):
                    nc.tensor.matmul(psq0[:], wq_sb[:, k * 256:k * 256 + 128],
                                     xap(k), start=(k == 0),
                                     stop=(k == NKT - 1))
                cq0 = tp.tile([128, QB], BF, tag="cq", name=f"cq0_{b}")
                nc.scalar.copy(cq0[:], psq0[:])
                psk = psB.tile([128, QB], F32, tag="B", name=f"k_{b}")
                for k in range(NKT):
                    nc.tensor.matmul(psk[:], wk_sb[:, k * 128:(k + 1) * 128],
                                     xap(k), start=(k == 0),
                                     stop=(k == NKT - 1))
                if b == 0:
                    # wv DMAs ride the Act queue after cast-q0(0)
                    for k in range(NKT):
                        nc.scalar.dma_start(wv_sb[:, k * 128:(k + 1) * 128],
                                            wvT[128 * k:128 * (k + 1), :])
                psq1 = psA.tile([128, QB], F32, tag="A", name=f"q1_{b}")
                for k in range(NKT):
                    nc.tensor.matmul(psq1[:],
                                     wq_sb[:, k * 256 + 128:k * 256 + 256],
                                     xap(k), start=(k == 0),
                                     stop=(k == NKT - 1))
                cq1 = tp.tile([128, QB], BF, tag="cq", name=f"cq1_{b}")
                nc.scalar.copy(cq1[:], psq1[:])
                psv = psB.tile([128, QB], F32, tag="B", name=f"v_{b}")
                for k in range(NKT):
                    nc.tensor.matmul(psv[:], wv_sb[:, k * 128:(k + 1) * 128],
                                     xap(k), start=(k == 0),
                                     stop=(k == NKT - 1))
                if b == 0:
                    load_x2(2)
                ps[b].update(psk=psk, psv=psv, cq0=cq0, cq1=cq1)

            def rope(dst, src, b, nm):
                """dst = src*cos2 + swap(src)*sin2; t on Pool (no partition
                shift allowed there), shifted u halves + add on DVE."""
                s0 = QB * b
                t = tp.tile([128, QB], BF, tag="rt", name=f"rt{nm}")
                u = tp.tile([128, QB], BF, tag="ru", name=f"ru{nm}")
                nc.gpsimd.tensor_tensor(t[:], src[:], cosk_s[:, s0:s0 + QB],
                                        MUL)
                nc.vector.tensor_tensor(u[0:64, :], src[64:128, :],
                                        sink_s[64:128, s0:s0 + QB], MUL)
                nc.vector.tensor_tensor(u[64:128, :], src[0:64, :],
                                        sink_s[0:64, s0:s0 + QB], MUL)
                nc.gpsimd.tensor_tensor(dst[:], t[:], u[:], ADD)

            def chains(b):
                """SSM + rope + gathers + vc transpose for block b. The k/v
                psum tiles are cast to SBUF (Act) so scan/STT can run on
                gpsimd (which cannot access PSUM), freeing DVE."""
                s0 = QB * b
                psk, psv = ps[b]['psk'], ps[b]['psv']
                # K chain first: feeds the next scores soonest (scan/STT are
                # DVE-only ops; gpsimd tensor_copy lowers to sw-DGE DMA)
                hk = tp.tile([128, QB], F32, tag="hk", name=f"hk{b}")
                nc.vector.tensor_tensor_scan(hk[:], akp_s[:], psk[:], 0.0,
                                             MUL, ADD)
                kp = tp.tile([128, QB], BF, tag="kp", name=f"kp{b}")
                nc.vector.scalar_tensor_tensor(kp[:], hk[:], cbk_s[:], psk[:],
                                               MUL, ADD)
                rope(KTp[:, s0:s0 + QB], kp, b, f"k{b}")
                nc.gpsimd.tensor_copy(KCt[:, 64 * b:64 * (b + 1)],
                                      KTp[:, s0 + 7:s0 + QB:8])
                if b == 0:
                    nc.gpsimd.tensor_copy(KCt[:, 256:260], KTp[:, 0:4])
                # V chain
                hv = tp.tile([128, QB], F32, tag="hk", name=f"hv{b}")
                nc.vector.tensor_tensor_scan(hv[:], avp_s[:], psv[:], 0.0,
                                             MUL, ADD)
                nc.vector.scalar_tensor_tensor(VTp[:, s0:s0 + QB], hv[:],
                                               cbv_s[:], psv[:], MUL, ADD)
                nc.gpsimd.tensor_copy(VG[:, 64 * b:64 * (b + 1)],
                                      VTp[:, s0 + 7:s0 + QB:8])
                if b == 0:
                    nc.gpsimd.tensor_copy(VG[:, 256:260], VTp[:, 0:4])
                # transpose V gathers pairwise into the hd-major stacks: one
                # [128,128] transpose per stack keeps every AP at base 0
                if b % 2 == 1:
                    pstf = psC.tile([128, 128], BF, tag="C",
                                    name=f"vt{b // 2}")
                    nc.tensor.transpose(pstf[:], VG[:, 128 * (b // 2):
                                                    128 * (b // 2) + 128],
                                        identb_s[:])
                    vdst = vstk0 if b < 2 else vstk1
                    nc.vector.tensor_copy(vdst[:], pstf[:])
                if b == 0:
                    pst4 = scp.tile([4, 128], BF, tag="sc", name="vt4")
                    nc.tensor.transpose(pst4[:], VG[:, 256:260], identb_s[:])
                    nc.vector.tensor_copy(vc4[:], pst4[:])
                # Q ropes
                rope(QT0[:, s0:s0 + QB], ps[b]['cq0'], b, f"q0{b}")
                rope(QT1[:, s0:s0 + QB], ps[b]['cq1'], b, f"q1{b}")

            # score stack column ranges per block
            STACKS = {0: [(0, 64)], 1: [(0, 128)],
                      2: [(0, 128), (128, 192)], 3: [(0, 128), (128, 256)]}

            def scores(b):
                """Score matmuls + exp for both heads; paired diag chain."""
                s0 = QB * b
                # diag elementwise products (DVE) before sd matmuls
                zs = []
                for h, QTh in enumerate((QT0, QT1)):
                    z = tp.tile([128, QB], BF, tag="z", name=f"z{b}_{h}")
                    if b == 0:
                        nc.vector.memset(z[:, 0:1], 0.0)
                        nc.vector.tensor_tensor(z[:, 1:QB], QTh[:, 1:QB],
                                                KTp[:, 0:QB - 1], MUL)
                    else:
                        nc.vector.tensor_tensor(z[:], QTh[:, s0:s0 + QB],
                                                KTp[:, s0 - 1:s0 + QB - 1],
                                                MUL)
                    zs.append(z)
                plist = [[], []]  # per head: (P, rows, vstack lhsT ap)
                pds = []
                for h, QTh in enumerate((QT0, QT1)):
                    for si, (c0, c1) in enumerate(STACKS[b]):
                        rows = c1 - c0
                        st = scp.tile([rows, QB], F32, tag="sc",
                                      name=f"st{b}_{h}_{si}")
                        nc.tensor.matmul(st[:], KCt[:, c0:c1],
                                         QTh[:, s0:s0 + QB],
                                         start=True, stop=True)
                        P = tp.tile([rows, QB], BF, tag="P", bufs=6,
                                    name=f"P{b}_{h}_{si}")
                        nc.scalar.activation(P[:], st[:], EXP, scale=SCALE)
                        # staircase mask on the current block's boundary rows
                        if si == len(STACKS[b]) - 1:
                            r0 = 64 * b - c0
                            nc.gpsimd.tensor_tensor(P[r0:r0 + 64, :],
                                                    P[r0:r0 + 64, :],
                                                    bandm01_s[r0:r0 + 64, :],
                                                    MUL)
                        vs = vstk0 if c0 == 0 else vstk1
                        plist[h].append((P, rows, vs[0:rows, :]))
                    # first-4 keys
                    st4 = scp.tile([4, QB], F32, tag="sc", name=f"st4_{b}_{h}")
                    nc.tensor.matmul(st4[:], KCt[:, 256:260],
                                     QTh[:, s0:s0 + QB], start=True, stop=True)
                    P4 = tp.tile([4, QB], BF, tag="P4", name=f"P4_{b}_{h}")
                    nc.scalar.activation(P4[:], st4[:], EXP, scale=SCALE)
                    if b == 0:
                        nc.gpsimd.tensor_tensor(P4[:], P4[:], f4m01_s[:], MUL)
                    plist[h].append((P4, 4, vc4[:]))
                    # diag score row
                    sd = scp.tile([1, QB], F32, tag="sc", name=f"sd{b}_{h}")
                    nc.tensor.matmul(sd[:], onesb_s[:], zs[h][:],
                                     start=True, stop=True)
                    pd = tp.tile([1, QB], BF, tag="pdm", name=f"pd{b}_{h}")
                    nc.scalar.activation(pd[:], sd[:], EXP, scale=SCALE)
                    nc.vector.tensor_tensor(pd[:], pd[:],
                                            mdiag01_s[b][:], MUL)
                    pds.append(pd)
                ps[b].update(plist=plist, pds=pds)

            def tails(b):
                """den/oun accumulation, diag add, normalize -> OT."""
                s0 = QB * b
                plist, pds = ps[b]['plist'], ps[b]['pds']
                ouns, dens = [], []
                for h in range(2):
                    n = len(plist[h])
                    den = scp.tile([1, QB], F32, tag="sc", name=f"dn{b}_{h}")
                    for i, (P, rows, _) in enumerate(plist[h]):
                        nc.tensor.matmul(den[:], onesb_s[0:rows, :], P[:],
                                         start=(i == 0), stop=(i == n - 1))
                    oun = ounp.tile([128, QB], F32, tag="oun",
                                    name=f"oun{b}_{h}")
                    for i, (P, rows, vs) in enumerate(plist[h]):
                        nc.tensor.matmul(oun[:], vs, P[:],
                                         start=(i == 0), stop=(i == n - 1))
                    ouns.append(oun)
                    dens.append(den)
                recs = []
                for h in range(2):
                    nc.vector.tensor_tensor(dens[h][:], dens[h][:],
                                            pds[h][:], ADD)
                    rec = tp.tile([1, QB], F32, tag="rec", name=f"rc{b}_{h}")
                    nc.vector.reciprocal_approx_fast(rec[:], dens[h][:])
                    recs.append(rec)
                for h, OTh in enumerate((OT0, OT1)):
                    psb = scp.tile([128, QB], F32, tag="sc", name=f"pb{b}_{h}")
                    nc.tensor.matmul(psb[:], onesrowb_s[:], pds[h][:],
                                     start=True, stop=True)
                    rb = scp.tile([128, QB], F32, tag="sc", name=f"rb{b}_{h}")
                    nc.tensor.matmul(rb[:], onesrowf_s[:], recs[h][:],
                                     start=True, stop=True)
                    rbs = tp.tile([128, QB], F32, tag="rbs", name=f"rs{b}_{h}")
                    nc.scalar.copy(rbs[:], rb[:])
                    vsh = tp.tile([128, QB], F32, tag="vsh", name=f"vs{b}_{h}")
                    if b == 0:
                        nc.vector.memset(vsh[:, 0:1], 0.0)
                        nc.vector.tensor_tensor(vsh[:, 1:QB], psb[:, 1:QB],
                                                VTp[:, 0:QB - 1], MUL)
                    else:
                        nc.vector.tensor_tensor(vsh[:], psb[:],
                                                VTp[:, s0 - 1:s0 + QB - 1],
                                                MUL)
                    oun = ouns[h]
                    nc.vector.tensor_tensor(oun[:], oun[:], vsh[:], ADD)
                    nc.vector.tensor_tensor(OTh[:, s0:s0 + QB], oun[:],
                                            rbs[:], MUL)

            def wo(b):
                """Output projection for s-block b; copies on Pool/Act."""
                s0 = QB * b
                for d in range(NKT):
                    d0 = 128 * d
                    pool = (psA, psB, psC)[d % 3]
                    yp = pool.tile([128, QB], F32, tag=("A", "B", "C")[d % 3],
                                   name=f"yp{b}_{d}")
                    nc.tensor.matmul(yp[:], wo_sb0[:, d0:d0 + 128],
                                     OT0[:, s0:s0 + QB], start=True, stop=False)
                    nc.tensor.matmul(yp[:], wo_sb1[:, d0:d0 + 128],
                                     OT1[:, s0:s0 + QB], start=False, stop=True)
                    yt = tp.tile([128, QB], BF, tag="yt", bufs=6,
                                 name=f"yt{b}_{d}")
                    if d % 2 == 0:
                        nc.vector.tensor_copy(yt[:], yp[:])
                    else:
                        nc.scalar.copy(yt[:], yp[:])
                    nc.sync.dma_start(yT[d0:d0 + 128, s0:s0 + QB], yt[:])

            # ---- phase interleave: PE never starved ----
            proj(0)
            chains(0)
            proj(1)
            chains(1)
            scores(0)
            proj(2)
            tails(0)
            chains(2)
            proj(3)
            scores(1)
            chains(3)
            wo(0)
            tails(1)
            scores(2)
            wo(1)
            tails(2)
            scores(3)
            wo(2)
            tails(3)
            wo(3)

    nc.compile()
    return nc


def _softplus(x):
    return np.log1p(np.exp(-np.abs(x))) + np.maximum(x, 0)


def _host_prep(inputs):
    import ml_dtypes
    bf16 = ml_dtypes.bfloat16

    x = np.asarray(inputs['x'], np.float32)
    freq = np.asarray(inputs['freq_cis'], np.float32)
    wq = np.asarray(inputs['wq'], np.float32)
    wk = np.asarray(inputs['wk'], np.float32)
    wv = np.asarray(inputs['wv'], np.float32)
    wo = np.asarray(inputs['wo'], np.float32)
    alk = np.asarray(inputs['a_log_k'], np.float32)
    bk = np.asarray(inputs['b_k'], np.float32)
    ck = np.asarray(inputs['c_k'], np.float32)
    alv = np.asarray(inputs['a_log_v'], np.float32)
    bv = np.asarray(inputs['b_v'], np.float32)
    cv = np.asarray(inputs['c_v'], np.float32)

    perm = np.concatenate([np.arange(0, HD, 2), np.arange(1, HD, 2)])
    xT = np.ascontiguousarray(x[0].T).astype(bf16)         # (D, S)
    cos = np.ascontiguousarray(freq[:, :, 0, 0].T)         # (64, S)
    sin = np.ascontiguousarray(freq[:, :, 1, 0].T)

    # multiplicative post-exp masks (staircase rows duplicated to 128
    # partitions so both 64-row halves are partition-aligned slices)
    bandm01 = np.zeros((64, QB), np.float32)
    for r in range(64):
        bandm01[r, 8 * r + 8:] = 1.0
    bandm01 = np.concatenate([bandm01, bandm01], 0)
    f4m01 = np.zeros((4, QB), np.float32)
    for k in range(4):
        f4m01[k, k:] = 1.0
    t = np.arange(S)
    mdiag01 = np.where((t >= 5) & (t % 8 != 0), 1.0, 0.0).astype(np.float32)
    mdiag01 = mdiag01.reshape(4, QB)

    cos2 = np.concatenate([cos, cos], 0).astype(bf16)
    sin2 = np.concatenate([sin, -sin], 0).astype(bf16)
    shared = {
        "xT": xT,
        "cosk": cos2, "sink": sin2,
        "bandm01": bandm01.astype(bf16), "f4m01": f4m01.astype(bf16),
        "mdiag01": mdiag01.astype(bf16),
        "identb": np.eye(128, dtype=np.float32).astype(bf16),
        "onesb": np.ones((128, 1), np.float32).astype(bf16),
        "onesrowb": np.ones((1, 128), np.float32).astype(bf16),
        "onesrowf": np.ones((1, 128), np.float32),
    }

    ak_full = np.exp(-_softplus(alk.astype(np.float64))).astype(np.float32)
    av_full = np.exp(-_softplus(alv.astype(np.float64))).astype(np.float32)
    col = np.arange(QB)

    in_maps = []
    for c in range(8):
        g = c // 2
        wq_c = wq[256 * c:256 * (c + 1)]
        wq_cp = np.concatenate([wq_c[h * HD:(h + 1) * HD][perm]
                                for h in range(2)])
        wk_g = wk[128 * g:128 * (g + 1)][perm]
        wv_g = wv[128 * g:128 * (g + 1)]
        ak = ak_full[128 * g:128 * (g + 1)][perm]
        bk_g = bk[128 * g:128 * (g + 1)][perm]
        ck_g = ck[128 * g:128 * (g + 1)][perm]
        av_ = av_full[128 * g:128 * (g + 1)]
        bv_g = bv[128 * g:128 * (g + 1)]
        cv_g = cv[128 * g:128 * (g + 1)]
        akp = np.where(col[None, :] % KC == 0, 0.0,
                       ak[:, None]).astype(np.float32)
        avp = np.where(col[None, :] % KC == 0, 0.0,
                       av_[:, None]).astype(np.float32)
        m = dict(shared)
        m.update({
            "wqT": np.ascontiguousarray(wq_cp.T).astype(bf16),
            "wkT": np.ascontiguousarray(wk_g.T).astype(bf16),
            "wvT": np.ascontiguousarray(wv_g.T).astype(bf16),
            "woT0": np.ascontiguousarray(
                wo[:, 256 * c:256 * c + 128].T).astype(bf16),
            "woT1": np.ascontiguousarray(
                wo[:, 256 * c + 128:256 * (c + 1)].T).astype(bf16),
            "akp": akp, "avp": avp,
            "cbk": (ck_g * bk_g)[:, None].astype(np.float32),
            "cbv": (cv_g * bv_g)[:, None].astype(np.float32),
        })
        in_maps.append(m)
    return in_maps


def kernel(**inputs) -> np.ndarray:
    if 'nc' not in _CACHE:
        _CACHE['nc'] = _build_module()
    nc = _CACHE['nc']
    in_maps = _host_prep(inputs)
    res = bass_utils.run_bass_kernel_spmd(nc, in_maps, core_ids=list(range(8)),
                                          **_CACHE.get('run_kwargs', {}))
    _CACHE['last_result'] = res
    yT = res.results[0]["yT"].astype(np.float64)
    for c in range(1, 8):
        yT += res.results[c]["yT"]
    return np.ascontiguousarray(yT.T[None]).astype(np.float32)


# revision 34
# speedup vs baseline: 1.1453x; 1.1453x over previous
"""Trainium2 Bass kernel for nn_AttentiveSSM (sparse chunked attention + SSM).

Sharding (8 cores, tensor-parallel over heads):
  core c owns q-heads {2c, 2c+1} and kv-head c//2. Each core computes its
  Q/K/V projections from the full (transposed) x in bf16, runs the chunked
  SSM + RoPE, sparse attention against the compressed key set (chunk
  boundaries + first-4 + t-1 diagonal), and a partial output projection
  through its wo column slice. Host sums the 8 partial yT outputs.

v2: bf16 matmul pipeline, stacked score/exp tiles, paired-head diag chain,
PE-dense issue order (proj/scores/tails/wo interleave), PSUM bank plan:
  A: q0/q1 + wo ring   B: k/v + wo ring   C: vc transposes + wo ring
  sc ring(2): score stacks, st4, psb, rb   sm(1): sd/den pairs   oun(2)

Self-contained: hardcodes all shapes; no sibling imports.
"""
import sys
import numpy as np

sys.path.insert(0, '/opt/trn_rl_repo')

import concourse.bacc as bacc               # noqa: E402
import concourse.mybir as mybir             # noqa: E402
from concourse.tile import TileContext      # noqa: E402
from concourse import bass_utils            # noqa: E402
from concourse.alu_op_type import AluOpType # noqa: E402

# silence cloud artifact upload in traced runs
bass_utils.upload_artifacts = lambda tmpdir: tmpdir

S = 2048          # sequence
D = 2048          # model dim
HD = 128          # head dim
QB = 512          # query block
NSB = S // QB     # 4 s-blocks
NKT = D // 128    # 16 contraction tiles
KC = 8            # token chunk
SCALE = float(1.0 / np.sqrt(HD))

F32 = mybir.dt.float32
F32R = mybir.dt.float32r
BF = mybir.dt.bfloat16
MUL = AluOpType.mult
ADD = AluOpType.add
EXP = mybir.ActivationFunctionType.Exp

_CACHE = {}


def _build_module():
    nc = bacc.Bacc("TRN2", num_devices=8)

    def din(name, shape, dt):
        return nc.dram_tensor(name, list(shape), dt, kind="ExternalInput")

    xT = din("xT", (D, S), BF)
    wqT = din("wqT", (D, 256), BF)
    wkT = din("wkT", (D, 128), BF)
    wvT = din("wvT", (D, 128), BF)
    woT0 = din("woT0", (128, D), BF)
    woT1 = din("woT1", (128, D), BF)
    cosk = din("cosk", (128, S), BF)    # halves duplicated
    sink = din("sink", (128, S), BF)    # [sin; -sin]
    akp = din("akp", (128, QB), F32)    # scan decay pattern (0 at i%8==0)
    avp = din("avp", (128, QB), F32)
    cbk = din("cbk", (128, 1), F32)     # c*b fused SSM output scale
    cbv = din("cbv", (128, 1), F32)
    bandm01 = din("bandm01", (128, QB), BF)  # post-exp staircase mask, rows
                                             # duplicated so any 64-partition
                                             # slice is partition-aligned
    f4m01 = din("f4m01", (4, QB), BF)
    mdiag01 = din("mdiag01", (4, QB), BF)    # diag valid mask per block
    identb = din("identb", (128, 128), BF)
    onesb = din("onesb", (128, 1), BF)
    onesrowb = din("onesrowb", (1, 128), BF)
    onesrowf = din("onesrowf", (1, 128), F32)
    yT = nc.dram_tensor("yT", [D, S], BF, kind="ExternalOutput")

    with TileContext(nc) as tc:
        with (
            tc.tile_pool(name="const", bufs=1) as cp,
            tc.tile_pool(name="big", bufs=1) as bp,
            tc.tile_pool(name="xs", bufs=20) as xs,
            tc.tile_pool(name="tmp", bufs=2) as tp,
            tc.tile_pool(name="psA", bufs=1, space="PSUM") as psA,
            tc.tile_pool(name="psB", bufs=1, space="PSUM") as psB,
            tc.tile_pool(name="psC", bufs=1, space="PSUM") as psC,
            tc.tile_pool(name="scp", bufs=3, space="PSUM") as scp,
            tc.tile_pool(name="ounp", bufs=2, space="PSUM") as ounp,
        ):
            # ---- constant tiles ----
            def cload(name, shape, src, dt, eng):
                t = cp.tile(list(shape), dt, tag=name, name=name)
                eng.dma_start(t[:], src[:])
                return t

            # Act queue: wq then wk (wv issued later, after cast-q0(0))
            wq_sb = cp.tile([128, NKT * 256], BF, tag="wq")
            wk_sb = cp.tile([128, NKT * 128], BF, tag="wk")
            wv_sb = cp.tile([128, NKT * 128], BF, tag="wv")
            for k in range(NKT):
                nc.scalar.dma_start(wq_sb[:, k * 256:(k + 1) * 256],
                                    wqT[128 * k:128 * (k + 1), :])
            for k in range(NKT):
                nc.scalar.dma_start(wk_sb[:, k * 128:(k + 1) * 128],
                                    wkT[128 * k:128 * (k + 1), :])
            for k in range(0, NKT, 2):
                nc.scalar.dma_start(wv_sb[:, k * 128:(k + 1) * 128],
                                    wvT[128 * k:128 * (k + 1), :])
            # SP queue: SSM consts (x tiles issued below, first)
            # gpsimd queue: rope tables, wv odds, masks, ident, wo weights
            cosk_s = cload("cosk", (128, S), cosk, BF, nc.gpsimd)
            sink_s = cload("sink", (128, S), sink, BF, nc.gpsimd)
            for k in range(1, NKT, 2):
                nc.gpsimd.dma_start(wv_sb[:, k * 128:(k + 1) * 128],
                                    wvT[128 * k:128 * (k + 1), :])
            identb_s = cload("identb", (128, 128), identb, BF, nc.gpsimd)
            onesb_s = cload("onesb", (128, 1), onesb, BF, nc.gpsimd)
            onesrowb_s = cload("onesrowb", (1, 128), onesrowb, BF, nc.gpsimd)
            onesrowf_s = cload("onesrowf", (1, 128), onesrowf, F32, nc.gpsimd)
            bandm01_s = cload("bandm01", (128, QB), bandm01, BF, nc.gpsimd)
            f4m01_s = cload("f4m01", (4, QB), f4m01, BF, nc.gpsimd)
            mdiag01_s = [cload(f"mdiag01_{b}", (1, QB), mdiag01[b:b + 1, :],
                               BF, nc.gpsimd) for b in range(4)]
            wo_sb0 = cload("wo0", (128, D), woT0, BF, nc.gpsimd)
            wo_sb1 = cload("wo1", (128, D), woT1, BF, nc.gpsimd)

            # ---- big persistent state ----
            QT0 = bp.tile([128, S], BF, tag="QT0")
            QT1 = bp.tile([128, S], BF, tag="QT1")
            KTp = bp.tile([128, S], BF, tag="KTp")
            VTp = bp.tile([128, S], BF, tag="VTp")
            OT0 = bp.tile([128, S], BF, tag="OT0")
            OT1 = bp.tile([128, S], BF, tag="OT1")
            KCt = bp.tile([128, 260], BF, tag="KCt")   # [b0|b1|b2|b3|first4]
            VG = bp.tile([128, 260], BF, tag="VG")
            vstk0 = bp.tile([128, 128], BF, tag="vstk0")  # keys 0:128 (hd-major)
            vstk1 = bp.tile([128, 128], BF, tag="vstk1")  # keys 128:256
            vc4 = bp.tile([4, 128], BF, tag="vc4")

            # x tile handles per block: (tile, col offset)
            xtiles = [[None] * NKT for _ in range(NSB)]

            def load_x(b):
                s0 = QB * b
                for k in range(NKT):
                    xt = xs.tile([128, QB], BF, tag="x", name=f"x{b}_{k}")
                    nc.sync.dma_start(xt[:], xT[128 * k:128 * (k + 1),
                                                s0:s0 + QB])
                    xtiles[b][k] = (xt, 0)

            # SP queue: x(0) head start, SSM consts, rest of x(0), x(1)
            for k in range(4):
                xt = xs.tile([128, QB], BF, tag="x", name=f"x0_{k}")
                nc.sync.dma_start(xt[:], xT[128 * k:128 * (k + 1), 0:QB])
                xtiles[0][k] = (xt, 0)
            akp_s = cload("akp", (128, QB), akp, F32, nc.sync)
            avp_s = cload("avp", (128, QB), avp, F32, nc.sync)
            cbk_s = cload("cbk", (128, 1), cbk, F32, nc.sync)
            cbv_s = cload("cbv", (128, 1), cbv, F32, nc.sync)
            for k in range(4, NKT):
                xt = xs.tile([128, QB], BF, tag="x", name=f"x0_{k}")
                nc.sync.dma_start(xt[:], xT[128 * k:128 * (k + 1), 0:QB])
                xtiles[0][k] = (xt, 0)
            load_x(1)

            # per-block psum/sbuf handles threaded between phases
            ps = [dict() for _ in range(NSB)]

            def proj(b):
                """QKV projection matmuls + q casts. PE order: q0, k, q1, v.
                Bank A: q0 then q1 (q1 waits cast-q0). Bank B: k then v
                (v waits scan/stt-k reading psum k)."""
                def xap(k):
                    xt, off = xtiles[b][k]
                    return xt[:, off:off + QB]
                psq0 = psA.tile([128, QB], F32, tag="A", name=f"q0_{b}")
                for k in range(NKT):
                    nc.tensor.matmul(psq0[:], wq_sb[:, k * 256:k * 256 + 128],
                                     xap(k), start=(k == 0),
                                     stop=(k == NKT - 1))
                cq0 = tp.tile([128, QB], BF, tag="cq", name=f"cq0_{b}")
                if b == 0:
                    nc.vector.tensor_copy(cq0[:], psq0[:])
                else:
                    nc.scalar.copy(cq0[:], psq0[:])
                psk = psB.tile([128, QB], F32, tag="B", name=f"k_{b}")
                for k in range(NKT):
                    nc.tensor.matmul(psk[:], wk_sb[:, k * 128:(k + 1) * 128],
                                     xap(k), start=(k == 0),
                                     stop=(k == NKT - 1))

                psq1 = psA.tile([128, QB], F32, tag="A", name=f"q1_{b}")
                for k in range(NKT):
                    nc.tensor.matmul(psq1[:],
                                     wq_sb[:, k * 256 + 128:k * 256 + 256],
                                     xap(k), start=(k == 0),
                                     stop=(k == NKT - 1))
                cq1 = tp.tile([128, QB], BF, tag="cq", name=f"cq1_{b}")
                if b == 0:
                    nc.vector.tensor_copy(cq1[:], psq1[:])
                else:
                    nc.scalar.copy(cq1[:], psq1[:])
                psv = psB.tile([128, QB], F32, tag="B", name=f"v_{b}")
                for k in range(NKT):
                    nc.tensor.matmul(psv[:], wv_sb[:, k * 128:(k + 1) * 128],
                                     xap(k), start=(k == 0),
                                     stop=(k == NKT - 1))
                if b + 2 < NSB:
                    load_x(b + 2)
                ps[b].update(psk=psk, psv=psv, cq0=cq0, cq1=cq1)

            def rope(dst, src, b, nm):
                """dst = src*cos2 + swap(src)*sin2; t on Pool (no partition
                shift allowed there), shifted u halves + add on DVE."""
                s0 = QB * b
                t = tp.tile([128, QB], BF, tag="rt", name=f"rt{nm}")
                u = tp.tile([128, QB], BF, tag="ru", name=f"ru{nm}")
                nc.vector.tensor_tensor(t[:], src[:], cosk_s[:, s0:s0 + QB],
                                        MUL)
                nc.vector.tensor_tensor(u[0:64, :], src[64:128, :],
                                        sink_s[64:128, s0:s0 + QB], MUL)
                nc.vector.tensor_tensor(u[64:128, :], src[0:64, :],
                                        sink_s[0:64, s0:s0 + QB], MUL)
                nc.vector.tensor_tensor(dst[:], t[:], u[:], ADD)

            def chains(b):
                """SSM + rope + gathers + vc transpose for block b. The k/v
                psum tiles are cast to SBUF (Act) so scan/STT can run on
                gpsimd (which cannot access PSUM), freeing DVE."""
                s0 = QB * b
                psk, psv = ps[b]['psk'], ps[b]['psv']
                # K chain first: feeds the next scores soonest (scan/STT are
                # DVE-only ops; gpsimd tensor_copy lowers to sw-DGE DMA)
                hk = tp.tile([128, QB], F32, tag="hk", name=f"hk{b}")
                nc.vector.tensor_tensor_scan(hk[:], akp_s[:], psk[:], 0.0,
                                             MUL, ADD)
                kp = tp.tile([128, QB], BF, tag="kp", name=f"kp{b}")
                nc.vector.scalar_tensor_tensor(kp[:], hk[:], cbk_s[:], psk[:],
                                               MUL, ADD)
                rope(KTp[:, s0:s0 + QB], kp, b, f"k{b}")
                nc.gpsimd.tensor_copy(KCt[:, 64 * b:64 * (b + 1)],
                                      KTp[:, s0 + 7:s0 + QB:8])
                if b == 0:
                    nc.gpsimd.tensor_copy(KCt[:, 256:260], KTp[:, 0:4])
                # V chain
                hv = tp.tile([128, QB], F32, tag="hk", name=f"hv{b}")
                nc.vector.tensor_tensor_scan(hv[:], avp_s[:], psv[:], 0.0,
                                             MUL, ADD)
                nc.vector.scalar_tensor_tensor(VTp[:, s0:s0 + QB], hv[:],
                                               cbv_s[:], psv[:], MUL, ADD)
                nc.gpsimd.tensor_copy(VG[:, 64 * b:64 * (b + 1)],
                                      VTp[:, s0 + 7:s0 + QB:8])
                if b == 0:
                    nc.gpsimd.tensor_copy(VG[:, 256:260], VTp[:, 0:4])
                # Q ropes
                rope(QT0[:, s0:s0 + QB], ps[b]['cq0'], b, f"q0{b}")
                rope(QT1[:, s0:s0 + QB], ps[b]['cq1'], b, f"q1{b}")

            # score stack column ranges per block
            STACKS = {0: [(0, 64)], 1: [(0, 128)],
                      2: [(0, 128), (128, 192)], 3: [(0, 128), (128, 256)]}

            def scores(b):
                """Score matmuls + exp for both heads; paired diag chain."""
                s0 = QB * b
                # diag elementwise products (DVE) before sd matmuls
                zs = []
                for h, QTh in enumerate((QT0, QT1)):
                    z = tp.tile([128, QB], BF, tag="z", name=f"z{b}_{h}")
                    if b == 0:
                        nc.vector.memset(z[:, 0:1], 0.0)
                        nc.vector.tensor_tensor(z[:, 1:QB], QTh[:, 1:QB],
                                                KTp[:, 0:QB - 1], MUL)
                    else:
                        nc.vector.tensor_tensor(z[:], QTh[:, s0:s0 + QB],
                                                KTp[:, s0 - 1:s0 + QB - 1],
                                                MUL)
                    zs.append(z)
                plist = [[], []]  # per head: (P, rows, vstack lhsT ap)
                pds = []
                for h, QTh in enumerate((QT0, QT1)):
                    for si, (c0, c1) in enumerate(STACKS[b]):
                        rows = c1 - c0
                        st = scp.tile([rows, QB], F32, tag="sc",
                                      name=f"st{b}_{h}_{si}")
                        nc.tensor.matmul(st[:], KCt[:, c0:c1],
                                         QTh[:, s0:s0 + QB],
                                         start=True, stop=True)
                        P = tp.tile([rows, QB], BF, tag="P", bufs=6,
                                    name=f"P{b}_{h}_{si}")
                        nc.scalar.activation(P[:], st[:], EXP, scale=SCALE)
                        # staircase mask on the current block's boundary rows
                        if si == len(STACKS[b]) - 1:
                            r0 = 64 * b - c0
                            nc.vector.tensor_tensor(P[r0:r0 + 64, :],
                                                    P[r0:r0 + 64, :],
                                                    bandm01_s[r0:r0 + 64, :],
                                                    MUL)
                        vs = vstk0 if c0 == 0 else vstk1
                        plist[h].append((P, rows, vs[0:rows, :]))
                    # first-4 keys
                    st4 = scp.tile([4, QB], F32, tag="sc", name=f"st4_{b}_{h}")
                    nc.tensor.matmul(st4[:], KCt[:, 256:260],
                                     QTh[:, s0:s0 + QB], start=True, stop=True)
                    P4 = tp.tile([4, QB], BF, tag="P4", name=f"P4_{b}_{h}")
                    nc.scalar.activation(P4[:], st4[:], EXP, scale=SCALE)
                    if b == 0:
                        nc.vector.tensor_tensor(P4[:], P4[:], f4m01_s[:], MUL)
                    plist[h].append((P4, 4, vc4[:]))
                    # diag score row
                    sd = scp.tile([1, QB], F32, tag="sc", name=f"sd{b}_{h}")
                    nc.tensor.matmul(sd[:], onesb_s[:], zs[h][:],
                                     start=True, stop=True)
                    pd = tp.tile([1, QB], BF, tag="pdm", name=f"pd{b}_{h}")
                    nc.scalar.activation(pd[:], sd[:], EXP, scale=SCALE)
                    nc.vector.tensor_tensor(pd[:], pd[:],
                                            mdiag01_s[b][:], MUL)
                    pds.append(pd)
                ps[b].update(plist=plist, pds=pds)

            def tails(b):
                """den/oun accumulation, diag add, normalize -> OT."""
                s0 = QB * b
                # pairwise V-stack transposes issued at the head of the tails
                # group so the PE reaches them well after the gpsimd gathers
                if b in (0, 2):
                    pstf = psC.tile([128, 128], BF, tag="C",
                                    name=f"vt{b // 2}")
                    nc.tensor.transpose(pstf[:], VG[:, 128 * (b // 2):
                                                    128 * (b // 2) + 128],
                                        identb_s[:])
                    vdst = vstk0 if b == 0 else vstk1
                    nc.vector.tensor_copy(vdst[:], pstf[:])
                if b == 0:
                    pst4 = scp.tile([4, 128], BF, tag="sc", name="vt4")
                    nc.tensor.transpose(pst4[:], VG[:, 256:260], identb_s[:])
                    nc.vector.tensor_copy(vc4[:], pst4[:])
                plist, pds = ps[b]['plist'], ps[b]['pds']
                ouns, dens = [], []
                for h in range(2):
                    n = len(plist[h])
                    den = scp.tile([1, QB], F32, tag="sc", name=f"dn{b}_{h}")
                    for i, (P, rows, _) in enumerate(plist[h]):
                        nc.tensor.matmul(den[:], onesb_s[0:rows, :], P[:],
                                         start=(i == 0), stop=(i == n - 1))
                    oun = ounp.tile([128, QB], F32, tag="oun",
                                    name=f"oun{b}_{h}")
                    for i, (P, rows, vs) in enumerate(plist[h]):
                        nc.tensor.matmul(oun[:], vs, P[:],
                                         start=(i == 0), stop=(i == n - 1))
                    ouns.append(oun)
                    dens.append(den)
                recs = []
                for h in range(2):
                    nc.vector.tensor_tensor(dens[h][:], dens[h][:],
                                            pds[h][:], ADD)
                    rec = tp.tile([1, QB], F32, tag="rec", name=f"rc{b}_{h}")
                    nc.vector.reciprocal_approx_fast(rec[:], dens[h][:])
                    recs.append(rec)
                for h, OTh in enumerate((OT0, OT1)):
                    psb = scp.tile([128, QB], F32, tag="sc", name=f"pb{b}_{h}")
                    nc.tensor.matmul(psb[:], onesrowb_s[:], pds[h][:],
                                     start=True, stop=True)
                    rb = scp.tile([128, QB], F32, tag="sc", name=f"rb{b}_{h}")
                    nc.tensor.matmul(rb[:], onesrowf_s[:], recs[h][:],
                                     start=True, stop=True)
                    rbs = tp.tile([128, QB], F32, tag="rbs", name=f"rs{b}_{h}")
                    nc.scalar.copy(rbs[:], rb[:])
                    vsh = tp.tile([128, QB], F32, tag="vsh", name=f"vs{b}_{h}")
                    if b == 0:
                        nc.vector.memset(vsh[:, 0:1], 0.0)
                        nc.vector.tensor_tensor(vsh[:, 1:QB], psb[:, 1:QB],
                                                VTp[:, 0:QB - 1], MUL)
                    else:
                        nc.vector.tensor_tensor(vsh[:], psb[:],
                                                VTp[:, s0 - 1:s0 + QB - 1],
                                                MUL)
                    oun = ouns[h]
                    nc.vector.tensor_tensor(oun[:], oun[:], vsh[:], ADD)
                    nc.vector.tensor_tensor(OTh[:, s0:s0 + QB], oun[:],
                                            rbs[:], MUL)

            def wo(b):
                """Output projection for s-block b; copies on Pool/Act."""
                s0 = QB * b
                for d in range(NKT):
                    d0 = 128 * d
                    pool = (psA, psB, psC)[d % 3]
                    yp = pool.tile([128, QB], F32, tag=("A", "B", "C")[d % 3],
                                   name=f"yp{b}_{d}")
                    nc.tensor.matmul(yp[:], wo_sb0[:, d0:d0 + 128],
                                     OT0[:, s0:s0 + QB], start=True, stop=False)
                    nc.tensor.matmul(yp[:], wo_sb1[:, d0:d0 + 128],
                                     OT1[:, s0:s0 + QB], start=False, stop=True)
                    yt = tp.tile([128, QB], BF, tag="yt", bufs=6,
                                 name=f"yt{b}_{d}")
                    if d % 2 == 0:
                        nc.vector.tensor_copy(yt[:], yp[:])
                    else:
                        nc.scalar.copy(yt[:], yp[:])
                    eng = nc.sync if d % 2 == 0 else nc.scalar
                    eng.dma_start(yT[d0:d0 + 128, s0:s0 + QB], yt[:])

            # ---- phase interleave: PE never starved ----
            proj(0)
            chains(0)
            proj(1)
            chains(1)
            scores(0)
            proj(2)
            tails(0)
            chains(2)
            proj(3)
            scores(1)
            chains(3)
            wo(0)
            tails(1)
            scores(2)
            wo(1)
            tails(2)
            scores(3)
            wo(2)
            tails(3)
            wo(3)

    nc.compile()
    return nc


def _softplus(x):
    return np.log1p(np.exp(-np.abs(x))) + np.maximum(x, 0)


def _host_prep(inputs):
    import ml_dtypes
    bf16 = ml_dtypes.bfloat16

    x = np.asarray(inputs['x'], np.float32)
    freq = np.asarray(inputs['freq_cis'], np.float32)
    wq = np.asarray(inputs['wq'], np.float32)
    wk = np.asarray(inputs['wk'], np.float32)
    wv = np.asarray(inputs['wv'], np.float32)
    wo = np.asarray(inputs['wo'], np.float32)
    alk = np.asarray(inputs['a_log_k'], np.float32)
    bk = np.asarray(inputs['b_k'], np.float32)
    ck = np.asarray(inputs['c_k'], np.float32)
    alv = np.asarray(inputs['a_log_v'], np.float32)
    bv = np.asarray(inputs['b_v'], np.float32)
    cv = np.asarray(inputs['c_v'], np.float32)

    perm = np.concatenate([np.arange(0, HD, 2), np.arange(1, HD, 2)])
    xT = np.ascontiguousarray(x[0].T).astype(bf16)         # (D, S)
    cos = np.ascontiguousarray(freq[:, :, 0, 0].T)         # (64, S)
    sin = np.ascontiguousarray(freq[:, :, 1, 0].T)

    # multiplicative post-exp masks (staircase rows duplicated to 128
    # partitions so both 64-row halves are partition-aligned slices)
    bandm01 = np.zeros((64, QB), np.float32)
    for r in range(64):
        bandm01[r, 8 * r + 8:] = 1.0
    bandm01 = np.concatenate([bandm01, bandm01], 0)
    f4m01 = np.zeros((4, QB), np.float32)
    for k in range(4):
        f4m01[k, k:] = 1.0
    t = np.arange(S)
    mdiag01 = np.where((t >= 5) & (t % 8 != 0), 1.0, 0.0).astype(np.float32)
    mdiag01 = mdiag01.reshape(4, QB)

    cos2 = np.concatenate([cos, cos], 0).astype(bf16)
    sin2 = np.concatenate([sin, -sin], 0).astype(bf16)
    shared = {
        "xT": xT,
        "cosk": cos2, "sink": sin2,
        "bandm01": bandm01.astype(bf16), "f4m01": f4m01.astype(bf16),
        "mdiag01": mdiag01.astype(bf16),
        "identb": np.eye(128, dtype=np.float32).astype(bf16),
        "onesb": np.ones((128, 1), np.float32).astype(bf16),
        "onesrowb": np.ones((1, 128), np.float32).astype(bf16),
        "onesrowf": np.ones((1, 128), np.float32),
    }

    ak_full = np.exp(-_softplus(alk.astype(np.float64))).astype(np.float32)
    av_full = np.exp(-_softplus(alv.astype(np.float64))).astype(np.float32)
    col = np.arange(QB)

    in_maps = []
    for c in range(8):
        g = c // 2
        wq_c = wq[256 * c:256 * (c + 1)]
        wq_cp = np.concatenate([wq_c[h * HD:(h + 1) * HD][perm]
                                for h in range(2)])
        wk_g = wk[128 * g:128 * (g + 1)][perm]
        wv_g = wv[128 * g:128 * (g + 1)]
        ak = ak_full[128 * g:128 * (g + 1)][perm]
        bk_g = bk[128 * g:128 * (g + 1)][perm]
        ck_g = ck[128 * g:128 * (g + 1)][perm]
        av_ = av_full[128 * g:128 * (g + 1)]
        bv_g = bv[128 * g:128 * (g + 1)]
        cv_g = cv[128 * g:128 * (g + 1)]
        akp = np.where(col[None, :] % KC == 0, 0.0,
                       ak[:, None]).astype(np.float32)
        avp = np.where(col[None, :] % KC == 0, 0.0,
                       av_[:, None]).astype(np.float32)
        m = dict(shared)
        m.update({
            "wqT": np.ascontiguousarray(wq_cp.T).astype(bf16),
            "wkT": np.ascontiguousarray(wk_g.T).astype(bf16),
            "wvT": np.ascontiguousarray(wv_g.T).astype(bf16),
            "woT0": np.ascontiguousarray(
                wo[:, 256 * c:256 * c + 128].T).astype(bf16),
            "woT1": np.ascontiguousarray(
                wo[:, 256 * c + 128:256 * (c + 1)].T).astype(bf16),
            "akp": akp, "avp": avp,
            "cbk": (ck_g * bk_g)[:, None].astype(np.float32),
            "cbv": (cv_g * bv_g)[:, None].astype(np.float32),
        })
        in_maps.append(m)
    return in_maps


def kernel(**inputs) -> np.ndarray:
    if 'nc' not in _CACHE:
        _CACHE['nc'] = _build_module()
    nc = _CACHE['nc']
    in_maps = _host_prep(inputs)
    res = bass_utils.run_bass_kernel_spmd(nc, in_maps, core_ids=list(range(8)),
                                          **_CACHE.get('run_kwargs', {}))
    _CACHE['last_result'] = res
    yT = res.results[0]["yT"].astype(np.float64)
    for c in range(1, 8):
        yT += res.results[c]["yT"]
    return np.ascontiguousarray(yT.T[None]).astype(np.float32)


# revision 35
# speedup vs baseline: 1.1520x; 1.0058x over previous
"""Trainium2 Bass kernel for nn_AttentiveSSM (sparse chunked attention + SSM).

Sharding (8 cores, tensor-parallel over heads):
  core c owns q-heads {2c, 2c+1} and kv-head c//2. Each core computes its
  Q/K/V projections from the full (transposed) x in bf16, runs the chunked
  SSM + RoPE, sparse attention against the compressed key set (chunk
  boundaries + first-4 + t-1 diagonal), and a partial output projection
  through its wo column slice. Host sums the 8 partial yT outputs.

v2: bf16 matmul pipeline, stacked score/exp tiles, paired-head diag chain,
PE-dense issue order (proj/scores/tails/wo interleave), PSUM bank plan:
  A: q0/q1 + wo ring   B: k/v + wo ring   C: vc transposes + wo ring
  sc ring(2): score stacks, st4, psb, rb   sm(1): sd/den pairs   oun(2)

Self-contained: hardcodes all shapes; no sibling imports.
"""
import sys
import numpy as np

sys.path.insert(0, '/opt/trn_rl_repo')

import concourse.bacc as bacc               # noqa: E402
import concourse.mybir as mybir             # noqa: E402
from concourse.tile import TileContext      # noqa: E402
from concourse import bass_utils            # noqa: E402
from concourse.alu_op_type import AluOpType # noqa: E402

# silence cloud artifact upload in traced runs
bass_utils.upload_artifacts = lambda tmpdir: tmpdir

S = 2048          # sequence
D = 2048          # model dim
HD = 128          # head dim
QB = 512          # query block
NSB = S // QB     # 4 s-blocks
NKT = D // 128    # 16 contraction tiles
KC = 8            # token chunk
SCALE = float(1.0 / np.sqrt(HD))

F32 = mybir.dt.float32
F32R = mybir.dt.float32r
BF = mybir.dt.bfloat16
MUL = AluOpType.mult
ADD = AluOpType.add
EXP = mybir.ActivationFunctionType.Exp

_CACHE = {}


def _build_module():
    nc = bacc.Bacc("TRN2", num_devices=8)

    def din(name, shape, dt):
        return nc.dram_tensor(name, list(shape), dt, kind="ExternalInput")

    xT = din("xT", (D, S), BF)
    wqT = din("wqT", (D, 256), BF)
    wkT = din("wkT", (D, 128), BF)
    wvT = din("wvT", (D, 128), BF)
    woT0 = din("woT0", (128, D), BF)
    woT1 = din("woT1", (128, D), BF)
    cosk = din("cosk", (128, S), BF)    # halves duplicated
    sink = din("sink", (128, S), BF)    # [sin; -sin]
    akp = din("akp", (128, QB), F32)    # scan decay pattern (0 at i%8==0)
    avp = din("avp", (128, QB), F32)
    cbk = din("cbk", (128, 1), F32)     # c*b fused SSM output scale
    cbv = din("cbv", (128, 1), F32)
    bandm01 = din("bandm01", (128, QB), BF)  # post-exp staircase mask, rows
                                             # duplicated so any 64-partition
                                             # slice is partition-aligned
    f4m01 = din("f4m01", (4, QB), BF)
    mdiag01 = din("mdiag01", (4, QB), BF)    # diag valid mask per block
    identb = din("identb", (128, 128), BF)
    onesb = din("onesb", (128, 1), BF)
    onesrowb = din("onesrowb", (1, 128), BF)
    onesrowf = din("onesrowf", (1, 128), F32)
    yT = nc.dram_tensor("yT", [D, S], BF, kind="ExternalOutput")

    with TileContext(nc) as tc:
        with (
            tc.tile_pool(name="const", bufs=1) as cp,
            tc.tile_pool(name="big", bufs=1) as bp,
            tc.tile_pool(name="xs", bufs=20) as xs,
            tc.tile_pool(name="tmp", bufs=2) as tp,
            tc.tile_pool(name="psA", bufs=1, space="PSUM") as psA,
            tc.tile_pool(name="psB", bufs=1, space="PSUM") as psB,
            tc.tile_pool(name="psC", bufs=1, space="PSUM") as psC,
            tc.tile_pool(name="scp", bufs=3, space="PSUM") as scp,
            tc.tile_pool(name="ounp", bufs=2, space="PSUM") as ounp,
        ):
            # ---- constant tiles ----
            def cload(name, shape, src, dt, eng):
                t = cp.tile(list(shape), dt, tag=name, name=name)
                eng.dma_start(t[:], src[:])
                return t

            # Act queue: wq then wk (wv issued later, after cast-q0(0))
            wq_sb = cp.tile([128, NKT * 256], BF, tag="wq")
            wk_sb = cp.tile([128, NKT * 128], BF, tag="wk")
            wv_sb = cp.tile([128, NKT * 128], BF, tag="wv")
            for k in range(NKT):
                nc.scalar.dma_start(wq_sb[:, k * 256:(k + 1) * 256],
                                    wqT[128 * k:128 * (k + 1), :])
            for k in range(NKT):
                nc.scalar.dma_start(wk_sb[:, k * 128:(k + 1) * 128],
                                    wkT[128 * k:128 * (k + 1), :])
            for k in range(0, NKT, 2):
                nc.scalar.dma_start(wv_sb[:, k * 128:(k + 1) * 128],
                                    wvT[128 * k:128 * (k + 1), :])
            # SP queue: SSM consts (x tiles issued below, first)
            # gpsimd queue: rope tables, wv odds, masks, ident, wo weights
            cosk_s = cload("cosk", (128, S), cosk, BF, nc.gpsimd)
            sink_s = cload("sink", (128, S), sink, BF, nc.gpsimd)
            for k in range(1, NKT, 2):
                nc.gpsimd.dma_start(wv_sb[:, k * 128:(k + 1) * 128],
                                    wvT[128 * k:128 * (k + 1), :])
            identb_s = cload("identb", (128, 128), identb, BF, nc.gpsimd)
            onesb_s = cload("onesb", (128, 1), onesb, BF, nc.gpsimd)
            onesrowb_s = cload("onesrowb", (1, 128), onesrowb, BF, nc.gpsimd)
            onesrowf_s = cload("onesrowf", (1, 128), onesrowf, F32, nc.gpsimd)
            bandm01_s = cload("bandm01", (128, QB), bandm01, BF, nc.gpsimd)
            f4m01_s = cload("f4m01", (4, QB), f4m01, BF, nc.gpsimd)
            mdiag01_s = [cload(f"mdiag01_{b}", (1, QB), mdiag01[b:b + 1, :],
                               BF, nc.gpsimd) for b in range(4)]
            wo_sb0 = cload("wo0", (128, D), woT0, BF, nc.gpsimd)
            wo_sb1 = cload("wo1", (128, D), woT1, BF, nc.gpsimd)

            # ---- big persistent state ----
            QT0 = bp.tile([128, S], BF, tag="QT0")
            QT1 = bp.tile([128, S], BF, tag="QT1")
            KTp = bp.tile([128, S], BF, tag="KTp")
            VTp = bp.tile([128, S], BF, tag="VTp")
            OT0 = bp.tile([128, S], BF, tag="OT0")
            OT1 = bp.tile([128, S], BF, tag="OT1")
            KCt = bp.tile([128, 260], BF, tag="KCt")   # [b0|b1|b2|b3|first4]
            VG = bp.tile([128, 260], BF, tag="VG")
            vstk0 = bp.tile([128, 128], BF, tag="vstk0")  # keys 0:128 (hd-major)
            vstk1 = bp.tile([128, 128], BF, tag="vstk1")  # keys 128:256
            vc4 = bp.tile([4, 128], BF, tag="vc4")

            # x tile handles per block: (tile, col offset)
            xtiles = [[None] * NKT for _ in range(NSB)]

            def load_x(b, split=False):
                s0 = QB * b
                for k in range(NKT):
                    xt = xs.tile([128, QB], BF, tag="x", name=f"x{b}_{k}")
                    eng = nc.gpsimd if (split and k % 2 == 1) else nc.sync
                    eng.dma_start(xt[:], xT[128 * k:128 * (k + 1),
                                            s0:s0 + QB])
                    xtiles[b][k] = (xt, 0)

            # SP queue: x(0) head start, SSM consts, rest of x(0), x(1)
            for k in range(4):
                xt = xs.tile([128, QB], BF, tag="x", name=f"x0_{k}")
                nc.sync.dma_start(xt[:], xT[128 * k:128 * (k + 1), 0:QB])
                xtiles[0][k] = (xt, 0)
            akp_s = cload("akp", (128, QB), akp, F32, nc.sync)
            avp_s = cload("avp", (128, QB), avp, F32, nc.sync)
            cbk_s = cload("cbk", (128, 1), cbk, F32, nc.sync)
            cbv_s = cload("cbv", (128, 1), cbv, F32, nc.sync)
            for k in range(4, NKT):
                xt = xs.tile([128, QB], BF, tag="x", name=f"x0_{k}")
                nc.sync.dma_start(xt[:], xT[128 * k:128 * (k + 1), 0:QB])
                xtiles[0][k] = (xt, 0)
            load_x(1)

            # per-block psum/sbuf handles threaded between phases
            ps = [dict() for _ in range(NSB)]

            def proj(b):
                """QKV projection matmuls + q casts. PE order: q0, k, q1, v.
                Bank A: q0 then q1 (q1 waits cast-q0). Bank B: k then v
                (v waits scan/stt-k reading psum k)."""
                def xap(k):
                    xt, off = xtiles[b][k]
                    return xt[:, off:off + QB]
                psq0 = psA.tile([128, QB], F32, tag="A", name=f"q0_{b}")
                for k in range(NKT):
                    nc.tensor.matmul(psq0[:], wq_sb[:, k * 256:k * 256 + 128],
                                     xap(k), start=(k == 0),
                                     stop=(k == NKT - 1))
                cq0 = tp.tile([128, QB], BF, tag="cq", name=f"cq0_{b}")
                if b == 0:
                    nc.vector.tensor_copy(cq0[:], psq0[:])
                else:
                    nc.scalar.copy(cq0[:], psq0[:])
                psk = psB.tile([128, QB], F32, tag="B", name=f"k_{b}")
                for k in range(NKT):
                    nc.tensor.matmul(psk[:], wk_sb[:, k * 128:(k + 1) * 128],
                                     xap(k), start=(k == 0),
                                     stop=(k == NKT - 1))

                psq1 = psA.tile([128, QB], F32, tag="A", name=f"q1_{b}")
                for k in range(NKT):
                    nc.tensor.matmul(psq1[:],
                                     wq_sb[:, k * 256 + 128:k * 256 + 256],
                                     xap(k), start=(k == 0),
                                     stop=(k == NKT - 1))
                cq1 = tp.tile([128, QB], BF, tag="cq", name=f"cq1_{b}")
                if b == 0:
                    nc.vector.tensor_copy(cq1[:], psq1[:])
                else:
                    nc.scalar.copy(cq1[:], psq1[:])
                psv = psB.tile([128, QB], F32, tag="B", name=f"v_{b}")
                for k in range(NKT):
                    nc.tensor.matmul(psv[:], wv_sb[:, k * 128:(k + 1) * 128],
                                     xap(k), start=(k == 0),
                                     stop=(k == NKT - 1))
                if b + 2 < NSB:
                    load_x(b + 2, split=True)
                ps[b].update(psk=psk, psv=psv, cq0=cq0, cq1=cq1)

            def rope(dst, src, b, nm):
                """dst = src*cos2 + swap(src)*sin2; t on Pool (no partition
                shift allowed there), shifted u halves + add on DVE."""
                s0 = QB * b
                t = tp.tile([128, QB], BF, tag="rt", name=f"rt{nm}")
                u = tp.tile([128, QB], BF, tag="ru", name=f"ru{nm}")
                nc.vector.tensor_tensor(t[:], src[:], cosk_s[:, s0:s0 + QB],
                                        MUL)
                nc.vector.tensor_tensor(u[0:64, :], src[64:128, :],
                                        sink_s[64:128, s0:s0 + QB], MUL)
                nc.vector.tensor_tensor(u[64:128, :], src[0:64, :],
                                        sink_s[0:64, s0:s0 + QB], MUL)
                nc.vector.tensor_tensor(dst[:], t[:], u[:], ADD)

            def chains(b):
                """SSM + rope + gathers + vc transpose for block b. The k/v
                psum tiles are cast to SBUF (Act) so scan/STT can run on
                gpsimd (which cannot access PSUM), freeing DVE."""
                s0 = QB * b
                psk, psv = ps[b]['psk'], ps[b]['psv']
                # K chain first: feeds the next scores soonest (scan/STT are
                # DVE-only ops; gpsimd tensor_copy lowers to sw-DGE DMA)
                hk = tp.tile([128, QB], F32, tag="hk", name=f"hk{b}")
                nc.vector.tensor_tensor_scan(hk[:], akp_s[:], psk[:], 0.0,
                                             MUL, ADD)
                kp = tp.tile([128, QB], BF, tag="kp", name=f"kp{b}")
                nc.vector.scalar_tensor_tensor(kp[:], hk[:], cbk_s[:], psk[:],
                                               MUL, ADD)
                rope(KTp[:, s0:s0 + QB], kp, b, f"k{b}")
                nc.gpsimd.tensor_copy(KCt[:, 64 * b:64 * (b + 1)],
                                      KTp[:, s0 + 7:s0 + QB:8])
                if b == 0:
                    nc.gpsimd.tensor_copy(KCt[:, 256:260], KTp[:, 0:4])
                # V chain
                hv = tp.tile([128, QB], F32, tag="hk", name=f"hv{b}")
                nc.vector.tensor_tensor_scan(hv[:], avp_s[:], psv[:], 0.0,
                                             MUL, ADD)
                nc.vector.scalar_tensor_tensor(VTp[:, s0:s0 + QB], hv[:],
                                               cbv_s[:], psv[:], MUL, ADD)
                nc.gpsimd.tensor_copy(VG[:, 64 * b:64 * (b + 1)],
                                      VTp[:, s0 + 7:s0 + QB:8])
                if b == 0:
                    nc.gpsimd.tensor_copy(VG[:, 256:260], VTp[:, 0:4])
                # Q ropes
                rope(QT0[:, s0:s0 + QB], ps[b]['cq0'], b, f"q0{b}")
                rope(QT1[:, s0:s0 + QB], ps[b]['cq1'], b, f"q1{b}")

            # score stack column ranges per block
            STACKS = {0: [(0, 64)], 1: [(0, 128)],
                      2: [(0, 128), (128, 192)], 3: [(0, 128), (128, 256)]}

            def scores(b):
                """Score matmuls + exp for both heads; paired diag chain."""
                s0 = QB * b
                # diag elementwise products (DVE) before sd matmuls
                zs = []
                for h, QTh in enumerate((QT0, QT1)):
                    z = tp.tile([128, QB], BF, tag="z", name=f"z{b}_{h}")
                    if b == 0:
                        nc.vector.memset(z[:, 0:1], 0.0)
                        nc.vector.tensor_tensor(z[:, 1:QB], QTh[:, 1:QB],
                                                KTp[:, 0:QB - 1], MUL)
                    else:
                        nc.vector.tensor_tensor(z[:], QTh[:, s0:s0 + QB],
                                                KTp[:, s0 - 1:s0 + QB - 1],
                                                MUL)
                    zs.append(z)
                plist = [[], []]  # per head: (P, rows, vstack lhsT ap)
                pds = []
                QTs = (QT0, QT1)
                # stack matmuls paired over heads: shared lhsT -> one
                # LDWEIGHTS per pair
                for si, (c0, c1) in enumerate(STACKS[b]):
                    rows = c1 - c0
                    for h in range(2):
                        st = scp.tile([rows, QB], F32, tag="sc",
                                      name=f"st{b}_{h}_{si}")
                        nc.tensor.matmul(st[:], KCt[:, c0:c1],
                                         QTs[h][:, s0:s0 + QB],
                                         start=True, stop=True)
                        P = tp.tile([rows, QB], BF, tag="P", bufs=6,
                                    name=f"P{b}_{h}_{si}")
                        nc.scalar.activation(P[:], st[:], EXP, scale=SCALE)
                        if si == len(STACKS[b]) - 1:
                            r0 = 64 * b - c0
                            nc.vector.tensor_tensor(P[r0:r0 + 64, :],
                                                    P[r0:r0 + 64, :],
                                                    bandm01_s[r0:r0 + 64, :],
                                                    MUL)
                        vs = vstk0 if c0 == 0 else vstk1
                        plist[h].append((P, rows, vs[0:rows, :]))
                # first-4 keys, head-paired
                P4s = []
                for h in range(2):
                    st4 = scp.tile([4, QB], F32, tag="sc", name=f"st4_{b}_{h}")
                    nc.tensor.matmul(st4[:], KCt[:, 256:260],
                                     QTs[h][:, s0:s0 + QB],
                                     start=True, stop=True)
                    P4 = tp.tile([4, QB], BF, tag="P4", name=f"P4_{b}_{h}")
                    nc.scalar.activation(P4[:], st4[:], EXP, scale=SCALE)
                    if b == 0:
                        nc.vector.tensor_tensor(P4[:], P4[:], f4m01_s[:], MUL)
                    P4s.append(P4)
                # diag rows, head-paired (shared ones lhsT)
                sds = [scp.tile([1, QB], F32, tag="sc", name=f"sd{b}_{h}")
                       for h in range(2)]
                for h in range(2):
                    nc.tensor.matmul(sds[h][:], onesb_s[:], zs[h][:],
                                     start=True, stop=True)
                for h in range(2):
                    plist[h].append((P4s[h], 4, vc4[:]))
                    pd = tp.tile([1, QB], BF, tag="pdm", name=f"pd{b}_{h}")
                    nc.scalar.activation(pd[:], sds[h][:], EXP, scale=SCALE)
                    nc.vector.tensor_tensor(pd[:], pd[:],
                                            mdiag01_s[b][:], MUL)
                    pds.append(pd)
                ps[b].update(plist=plist, pds=pds)

            def tails(b):
                """den/oun accumulation, diag add, normalize -> OT."""
                s0 = QB * b
                # pairwise V-stack transposes issued at the head of the tails
                # group so the PE reaches them well after the gpsimd gathers
                if b in (0, 2):
                    pstf = psC.tile([128, 128], BF, tag="C",
                                    name=f"vt{b // 2}")
                    nc.tensor.transpose(pstf[:], VG[:, 128 * (b // 2):
                                                    128 * (b // 2) + 128],
                                        identb_s[:])
                    vdst = vstk0 if b == 0 else vstk1
                    nc.vector.tensor_copy(vdst[:], pstf[:])
                if b == 0:
                    pst4 = scp.tile([4, 128], BF, tag="sc", name="vt4")
                    nc.tensor.transpose(pst4[:], VG[:, 256:260], identb_s[:])
                    nc.vector.tensor_copy(vc4[:], pst4[:])
                plist, pds = ps[b]['plist'], ps[b]['pds']
                n = len(plist[0])
                dens = [scp.tile([1, QB], F32, tag="sc", name=f"dn{b}_{h}")
                        for h in range(2)]
                ouns = [ounp.tile([128, QB], F32, tag="oun",
                                  name=f"oun{b}_{h}") for h in range(2)]
                # head-paired accumulation: each (stack, h0/h1) pair shares
                # its stationary lhsT
                for i in range(n):
                    rows = plist[0][i][1]
                    for h in range(2):
                        nc.tensor.matmul(dens[h][:], onesb_s[0:rows, :],
                                         plist[h][i][0][:],
                                         start=(i == 0), stop=(i == n - 1))
                for i in range(n):
                    for h in range(2):
                        P, rows, vs = plist[h][i]
                        nc.tensor.matmul(ouns[h][:], vs, P[:],
                                         start=(i == 0), stop=(i == n - 1))
                recs = []
                for h in range(2):
                    nc.vector.tensor_tensor(dens[h][:], dens[h][:],
                                            pds[h][:], ADD)
                    rec = tp.tile([1, QB], F32, tag="rec", name=f"rc{b}_{h}")
                    nc.vector.reciprocal_approx_fast(rec[:], dens[h][:])
                    recb = tp.tile([1, QB], BF, tag="recb", name=f"rcb{b}_{h}")
                    nc.scalar.copy(recb[:], rec[:])
                    recs.append(recb)
                psbs, rbs_t = [], []
                for h in range(2):
                    psb = scp.tile([128, QB], F32, tag="sc", name=f"pb{b}_{h}")
                    nc.tensor.matmul(psb[:], onesrowb_s[:], pds[h][:],
                                     start=True, stop=True)
                    psbs.append(psb)
                for h in range(2):
                    rb = scp.tile([128, QB], F32, tag="sc", name=f"rb{b}_{h}")
                    nc.tensor.matmul(rb[:], onesrowb_s[:], recs[h][:],
                                     start=True, stop=True)
                    rbs_t.append(rb)
                for h, OTh in enumerate((OT0, OT1)):
                    psb = psbs[h]
                    rbs = tp.tile([128, QB], F32, tag="rbs", name=f"rs{b}_{h}")
                    nc.scalar.copy(rbs[:], rbs_t[h][:])
                    vsh = tp.tile([128, QB], F32, tag="vsh", name=f"vs{b}_{h}")
                    if b == 0:
                        nc.vector.memset(vsh[:, 0:1], 0.0)
                        nc.vector.tensor_tensor(vsh[:, 1:QB], psb[:, 1:QB],
                                                VTp[:, 0:QB - 1], MUL)
                    else:
                        nc.vector.tensor_tensor(vsh[:], psb[:],
                                                VTp[:, s0 - 1:s0 + QB - 1],
                                                MUL)
                    oun = ouns[h]
                    nc.vector.tensor_tensor(oun[:], oun[:], vsh[:], ADD)
                    nc.vector.tensor_tensor(OTh[:, s0:s0 + QB], oun[:],
                                            rbs[:], MUL)

            def wo(b):
                """Output projection for s-block b; copies on Pool/Act."""
                s0 = QB * b
                for d in range(NKT):
                    d0 = 128 * d
                    pool = (psA, psB, psC)[d % 3]
                    yp = pool.tile([128, QB], F32, tag=("A", "B", "C")[d % 3],
                                   name=f"yp{b}_{d}")
                    nc.tensor.matmul(yp[:], wo_sb0[:, d0:d0 + 128],
                                     OT0[:, s0:s0 + QB], start=True, stop=False)
                    nc.tensor.matmul(yp[:], wo_sb1[:, d0:d0 + 128],
                                     OT1[:, s0:s0 + QB], start=False, stop=True)
                    yt = tp.tile([128, QB], BF, tag="yt", bufs=6,
                                 name=f"yt{b}_{d}")
                    if d % 2 == 0:
                        nc.vector.tensor_copy(yt[:], yp[:])
                    else:
                        nc.scalar.copy(yt[:], yp[:])
                    eng = (nc.sync, nc.scalar, nc.gpsimd)[d % 3]
                    eng.dma_start(yT[d0:d0 + 128, s0:s0 + QB], yt[:])

            # ---- phase interleave: PE never starved ----
            proj(0)
            chains(0)
            proj(1)
            chains(1)
            scores(0)
            proj(2)
            tails(0)
            chains(2)
            proj(3)
            scores(1)
            chains(3)
            wo(0)
            tails(1)
            scores(2)
            wo(1)
            tails(2)
            scores(3)
            wo(2)
            tails(3)
            wo(3)

    nc.compile()
    return nc


def _softplus(x):
    return np.log1p(np.exp(-np.abs(x))) + np.maximum(x, 0)


def _host_prep(inputs):
    import ml_dtypes
    bf16 = ml_dtypes.bfloat16

    x = np.asarray(inputs['x'], np.float32)
    freq = np.asarray(inputs['freq_cis'], np.float32)
    wq = np.asarray(inputs['wq'], np.float32)
    wk = np.asarray(inputs['wk'], np.float32)
    wv = np.asarray(inputs['wv'], np.float32)
    wo = np.asarray(inputs['wo'], np.float32)
    alk = np.asarray(inputs['a_log_k'], np.float32)
    bk = np.asarray(inputs['b_k'], np.float32)
    ck = np.asarray(inputs['c_k'], np.float32)
    alv = np.asarray(inputs['a_log_v'], np.float32)
    bv = np.asarray(inputs['b_v'], np.float32)
    cv = np.asarray(inputs['c_v'], np.float32)

    perm = np.concatenate([np.arange(0, HD, 2), np.arange(1, HD, 2)])
    xT = np.ascontiguousarray(x[0].T).astype(bf16)         # (D, S)
    cos = np.ascontiguousarray(freq[:, :, 0, 0].T)         # (64, S)
    sin = np.ascontiguousarray(freq[:, :, 1, 0].T)

    # multiplicative post-exp masks (staircase rows duplicated to 128
    # partitions so both 64-row halves are partition-aligned slices)
    bandm01 = np.zeros((64, QB), np.float32)
    for r in range(64):
        bandm01[r, 8 * r + 8:] = 1.0
    bandm01 = np.concatenate([bandm01, bandm01], 0)
    f4m01 = np.zeros((4, QB), np.float32)
    for k in range(4):
        f4m01[k, k:] = 1.0
    t = np.arange(S)
    mdiag01 = np.where((t >= 5) & (t % 8 != 0), 1.0, 0.0).astype(np.float32)
    mdiag01 = mdiag01.reshape(4, QB)

    cos2 = np.concatenate([cos, cos], 0).astype(bf16)
    sin2 = np.concatenate([sin, -sin], 0).astype(bf16)
    shared = {
        "xT": xT,
        "cosk": cos2, "sink": sin2,
        "bandm01": bandm01.astype(bf16), "f4m01": f4m01.astype(bf16),
        "mdiag01": mdiag01.astype(bf16),
        "identb": np.eye(128, dtype=np.float32).astype(bf16),
        "onesb": np.ones((128, 1), np.float32).astype(bf16),
        "onesrowb": np.ones((1, 128), np.float32).astype(bf16),
        "onesrowf": np.ones((1, 128), np.float32),
    }

    ak_full = np.exp(-_softplus(alk.astype(np.float64))).astype(np.float32)
    av_full = np.exp(-_softplus(alv.astype(np.float64))).astype(np.float32)
    col = np.arange(QB)

    in_maps = []
    for c in range(8):
        g = c // 2
        wq_c = wq[256 * c:256 * (c + 1)]
        wq_cp = np.concatenate([wq_c[h * HD:(h + 1) * HD][perm]
                                for h in range(2)])
        wk_g = wk[128 * g:128 * (g + 1)][perm]
        wv_g = wv[128 * g:128 * (g + 1)]
        ak = ak_full[128 * g:128 * (g + 1)][perm]
        bk_g = bk[128 * g:128 * (g + 1)][perm]
        ck_g = ck[128 * g:128 * (g + 1)][perm]
        av_ = av_full[128 * g:128 * (g + 1)]
        bv_g = bv[128 * g:128 * (g + 1)]
        cv_g = cv[128 * g:128 * (g + 1)]
        akp = np.where(col[None, :] % KC == 0, 0.0,
                       ak[:, None]).astype(np.float32)
        avp = np.where(col[None, :] % KC == 0, 0.0,
                       av_[:, None]).astype(np.float32)
        m = dict(shared)
        m.update({
            "wqT": np.ascontiguousarray(wq_cp.T).astype(bf16),
            "wkT": np.ascontiguousarray(wk_g.T).astype(bf16),
            "wvT": np.ascontiguousarray(wv_g.T).astype(bf16),
            "woT0": np.ascontiguousarray(
                wo[:, 256 * c:256 * c + 128].T).astype(bf16),
            "woT1": np.ascontiguousarray(
                wo[:, 256 * c + 128:256 * (c + 1)].T).astype(bf16),
            "akp": akp, "avp": avp,
            "cbk": (ck_g * bk_g)[:, None].astype(np.float32),
            "cbv": (cv_g * bv_g)[:, None].astype(np.float32),
        })
        in_maps.append(m)
    return in_maps


def kernel(**inputs) -> np.ndarray:
    if 'nc' not in _CACHE:
        _CACHE['nc'] = _build_module()
    nc = _CACHE['nc']
    in_maps = _host_prep(inputs)
    res = bass_utils.run_bass_kernel_spmd(nc, in_maps, core_ids=list(range(8)),
                                          **_CACHE.get('run_kwargs', {}))
    _CACHE['last_result'] = res
    yT = res.results[0]["yT"].astype(np.float64)
    for c in range(1, 8):
        yT += res.results[c]["yT"]
    return np.ascontiguousarray(yT.T[None]).astype(np.float32)
